# revision 1
# baseline (speedup 1.0000x reference)
"""MoE layer (16 experts, top-6 routing, H=1024) on 8 TRN2 NeuronCores.

Strategy: data-parallel over tokens. Each core takes a 1024-token chunk and
the full router/expert weights, computes routing + the dense weighted
expert-matmul sum for its chunk, no inter-core communication. Host just
splits tokens and concatenates outputs.

Math (algebraically identical to the reference):
  logits = x @ Rw^T + b                       (fp32 on PE)
  top6 selected on logits (softmax is monotone; selection on logits avoids
  exp-table rounding flipping membership)
  w_e = exp(l_e - max) * [l_e >= thr6] / S6   (renormalized top-6 gate; the
  softmax denominator cancels, and the reference's /acc renormalization
  collapses to exactly this)
  out = sum_e (x @ W_e) * w_e + 1e-10         (the EPS*sum(top_p) term equals
  EPS to within 3e-20)

Expert matmuls run in bf16 (full-speed PE, 1 cycle/row), accumulation in
fp32 PSUM, combine via one fused (psum*w + acc) DVE op per PSUM tile. The
f32->bf16 conversions run on the otherwise-idle ACT engine.
"""

import numpy as np
from contextlib import ExitStack

import concourse.bass as bass
import concourse.bacc as bacc
import concourse.mybir as mybir
import concourse.tile as tile
from concourse.masks import make_identity

P = 128
H = 1024
E = 16
T_CORE = 1024
N_CORES = 8
KT = H // P       # 8 contraction tiles
MT = T_CORE // P  # 8 token tiles per core
NH = 2            # h_out halves (free dim 512)
FD = 512
EPS = 1e-10
F32 = mybir.dt.float32
F32R = mybir.dt.float32r
BF16 = mybir.dt.bfloat16


def build_program(n_experts: int = E, t_core: int = T_CORE):
    mt = t_core // P
    nc = bacc.Bacc(None, target_bir_lowering=False)
    tokens = nc.dram_tensor("tokens", [t_core, H], F32, kind="ExternalInput")
    router_w = nc.dram_tensor("router_w", [n_experts, H], F32, kind="ExternalInput")
    router_b = nc.dram_tensor("router_b", [n_experts], F32, kind="ExternalInput")
    expert_w = nc.dram_tensor("expert_w", [n_experts, H, H], F32, kind="ExternalInput")
    out = nc.dram_tensor("out", [t_core, H], F32, kind="ExternalOutput")

    with tile.TileContext(nc) as tc, ExitStack() as ctx:
        persist = ctx.enter_context(tc.tile_pool(name="persist", bufs=1))
        xt = [persist.tile([P, t_core], F32, tag=f"xt{k}", name=f"xt{k}") for k in range(KT)]
        xtb = [persist.tile([P, t_core], BF16, tag=f"xtb{k}", name=f"xtb{k}") for k in range(KT)]
        acc = [persist.tile([P, H], F32, tag=f"acc{m}", name=f"acc{m}") for m in range(mt)]
        wg = [persist.tile([P, n_experts], F32, tag=f"wg{m}", name=f"wg{m}") for m in range(mt)]
        rt = persist.tile([P, KT, n_experts], F32, tag="rt")
        bias = persist.tile([P, n_experts], F32, tag="bias")
        ident = persist.tile([P, P], F32, tag="ident")

        make_identity(nc, ident[:])
        nc.sync.dma_start(bias[:], router_b[None, :].to_broadcast((P, n_experts)))

        wstage = ctx.enter_context(tc.tile_pool(name="wstage", bufs=6))
        wbpool = ctx.enter_context(tc.tile_pool(name="wbpool", bufs=2))

        with (
            tc.tile_pool(name="xpool", bufs=2) as xpool,
            tc.tile_pool(name="rwpool", bufs=1) as rwpool,
            tc.tile_pool(name="tpsum", bufs=2, space="PSUM") as tpsum,
            tc.tile_pool(name="small", bufs=3) as small,
        ):
            # router_w^T: [E,H] -> 8 PE transposes of [E,128] -> [128,E]
            rw = rwpool.tile([n_experts, H], F32, tag="rw")
            nc.sync.dma_start(rw[:], router_w[:])
            for k in range(KT):
                pt = tpsum.tile([P, n_experts], F32, tag="rtp")
                nc.tensor.transpose(
                    pt[:],
                    rw[:, k * P : (k + 1) * P],
                    ident[:n_experts, :n_experts],
                )
                nc.vector.tensor_copy(rt[:, k, :], pt[:])

            # X load + transpose into X^T tiles [h_in slice, tokens]
            for m in range(mt):
                xm = xpool.tile([P, H], F32, tag="xm")
                nc.sync.dma_start(xm[:], tokens[m * P : (m + 1) * P, :])
                for k in range(KT):
                    pt2 = tpsum.tile([P, P], F32, tag="xtp", bufs=4)
                    nc.tensor.transpose(pt2[:], xm[:, k * P : (k + 1) * P], ident[:])
                    nc.vector.tensor_copy(xt[k][:, m * P : (m + 1) * P], pt2[:])
                    nc.scalar.copy(xtb[k][:, m * P : (m + 1) * P], pt2[:])

            # Router + top-6 gate weights per token tile
            for m in range(mt):
                lps = tpsum.tile([P, n_experts], F32, tag="lps")
                for k in range(KT):
                    nc.tensor.matmul(
                        lps[:],
                        xt[k][:, m * P : (m + 1) * P],
                        rt[:, k, :],
                        start=(k == 0),
                        stop=(k == KT - 1),
                    )
                logits = small.tile([P, n_experts], F32, tag="logits")
                nc.vector.tensor_add(logits[:], lps[:], bias[:])
                mx8 = small.tile([P, 8], F32, tag="mx8")
                nc.vector.max(mx8[:], logits[:])
                nm = small.tile([P, 1], F32, tag="nm")
                nc.vector.tensor_scalar_mul(nm[:], mx8[:, 0:1], -1.0)
                expo = small.tile([P, n_experts], F32, tag="expo")
                nc.scalar.activation(
                    expo[:], logits[:], mybir.ActivationFunctionType.Exp,
                    bias=nm[:], scale=1.0,
                )
                wraw = small.tile([P, n_experts], F32, tag="wraw")
                s6 = small.tile([P, 1], F32, tag="s6")
                nc.vector.scalar_tensor_tensor(
                    wraw[:], logits[:], mx8[:, 5:6], expo[:],
                    op0=mybir.AluOpType.is_ge, op1=mybir.AluOpType.mult,
                    accum_out=s6[:],
                )
                r6 = small.tile([P, 1], F32, tag="r6")
                nc.vector.reciprocal(r6[:], s6[:])
                nc.vector.tensor_scalar_mul(wg[m][:], wraw[:], r6[:])
                nc.vector.memset(acc[m][:], EPS)

        # Dense expert loop: acc[m] += (X @ W_e) * w[:, e]
        with (
            tc.tile_pool(name="mmpsum", bufs=8, space="PSUM") as mmpsum,
        ):
            for e in range(n_experts):
                wtb = wbpool.tile([P, KT, H], BF16, tag="wtb")
                for k in range(KT):
                    ws = wstage.tile([P, H], F32, tag="ws", name=f"ws_{e}_{k}")
                    nc.sync.dma_start(ws[:], expert_w[e, k * P : (k + 1) * P, :])
                    nc.scalar.copy(wtb[:, k, :], ws[:])
                for m in range(mt):
                    for nh in range(NH):
                        ps = mmpsum.tile([P, FD], F32, tag="ps")
                        for k in range(KT):
                            nc.tensor.matmul(
                                ps[:],
                                xtb[k][:, m * P : (m + 1) * P],
                                wtb[:, k, nh * FD : (nh + 1) * FD],
                                start=(k == 0),
                                stop=(k == KT - 1),
                            )
                        nc.vector.scalar_tensor_tensor(
                            acc[m][:, nh * FD : (nh + 1) * FD],
                            ps[:],
                            wg[m][:, e : e + 1],
                            acc[m][:, nh * FD : (nh + 1) * FD],
                            op0=mybir.AluOpType.mult,
                            op1=mybir.AluOpType.add,
                        )

        # Epilogue: store (acc was initialized to EPS, covering the
        # reference's +EPS*sum(top_p) term to within ~1e-10 absolute)
        for m in range(mt):
            nc.sync.dma_start(out[m * P : (m + 1) * P, :], acc[m][:])

    nc.finalize()
    return nc


_PROGRAM_CACHE: dict = {}


def _get_program(n_experts: int = E, t_core: int = T_CORE):
    key = (n_experts, t_core)
    if key not in _PROGRAM_CACHE:
        _PROGRAM_CACHE[key] = build_program(n_experts, t_core)
    return _PROGRAM_CACHE[key]


def kernel(tokens: np.ndarray, router_w: np.ndarray, router_b: np.ndarray,
           expert_w: np.ndarray) -> np.ndarray:
    from concourse.bass_utils import run_bass_kernel_spmd

    B, S, hidden = tokens.shape
    T = B * S
    assert hidden == H and T == N_CORES * T_CORE

    x = np.ascontiguousarray(tokens.reshape(T, H), dtype=np.float32)
    rw = np.ascontiguousarray(router_w, dtype=np.float32)
    rb = np.ascontiguousarray(router_b, dtype=np.float32)
    ew = np.ascontiguousarray(expert_w, dtype=np.float32)

    nc = _get_program()
    in_maps = [
        {
            "tokens": x[c * T_CORE : (c + 1) * T_CORE],
            "router_w": rw,
            "router_b": rb,
            "expert_w": ew,
        }
        for c in range(N_CORES)
    ]
    res = run_bass_kernel_spmd(nc, in_maps, list(range(N_CORES)))
    out = np.concatenate([res.results[c]["out"] for c in range(N_CORES)], axis=0)
    return out.reshape(B, S, H).astype(np.float32)



# revision 7
# speedup vs baseline: 1.5601x; 1.5601x over previous
"""MoE layer (16 experts, top-6 routing, H=1024) on 8 TRN2 NeuronCores.

Strategy: data-parallel over tokens (1024 tokens/core), with SPARSE
expert compute — each token is processed only by its top-6 experts
(37.5% of the dense FLOPs) via on-device token dispatch:

  1. Router matmul (fp16 X^T tiles stationary) -> logits [128, 16] per
     token tile; top-6 selection on logits (softmax is monotone), gate
     weights renormalized over the top-6 (the reference's /acc
     renormalization collapses to exactly this).
  2. Per-expert token index lists built on-device: a selection mask in
     16-partition "wrapped" layout (8 small rearrange DMAs), then one
     gpsimd sparse_gather per expert (stream compaction), count
     clamped to capacity C=448.
  3. Per expert: dma_gather(transpose=True) pulls the selected token
     rows from DRAM as X^T tiles [128, k, C]; fp16 matmuls with the
     gathered tokens stationary and W_e moving accumulate Y = X_g @ W_e
     token-major in PSUM; DVE scales rows by the gathered gate weight;
     dma_scatter_add accumulates the scaled rows into the fp16 output
     at the token's row.

Tokens/weights/output run in fp16 (PE full speed, ~1e-4 matmul noise);
gates in fp32. Expert capacity C=448 (~4σ above the mean count of 384;
overflow tokens would be dropped gracefully). Host pre-layouts inputs
(transposes/dtype casts) and un-permutes the output; all routing math
and all FLOPs are on-device.
"""

import numpy as np
from contextlib import ExitStack

import concourse.bass as bass
import concourse.bacc as bacc
import concourse.mybir as mybir
import concourse.tile as tile

P = 128
H = 1024
E = 16
KT = 8            # h_in slices of 128
MT = 8            # token tiles of 128 per core
T_CORE = 1024
N_CORES = 8
CDMA = 512        # dma_gather transpose capacity (multiple of 128)
CMM = 448         # compute/scatter capacity (28 wrapped cols * 16)
CW = CMM // 16    # wrapped columns = 28
F16 = mybir.dt.float16
F32 = mybir.dt.float32
I16 = mybir.dt.int16
U32 = mybir.dt.uint32
POOL_E = mybir.EngineType.Pool


def build_program():
    nc = bacc.Bacc(None, target_bir_lowering=False)
    # Host-prepared layouts (see kernel() below):
    #   xt[p, k, t]   = x[t, 128k + p]          fp16 (router stationary)
    #   xbf[r, h]     = x[tok_of_r[r], h]       fp16 (dispatch gather rows)
    #   rwt[p, k, e]  = router_w[e, 128k + p]   fp16
    #   rb[e]                                   fp32
    #   wd[e, p, k, h] = expert_w[e, 128k + p, h] fp16
    #   out[r, h]     (r-numbered rows; host un-permutes)
    xt = nc.dram_tensor("xt", [P, KT, T_CORE], F32, kind="ExternalInput")
    xbf = nc.dram_tensor("xbf", [T_CORE, H], F16, kind="ExternalInput")
    rwt = nc.dram_tensor("rwt", [P, KT, E], F32, kind="ExternalInput")
    rb = nc.dram_tensor("rb", [E], F32, kind="ExternalInput")
    wd = nc.dram_tensor("wd", [E, P, KT, H], F16, kind="ExternalInput")
    gd = nc.dram_tensor("gd", [T_CORE, 64], F32, kind="Internal")
    nfd = nc.dram_tensor("nfd", [1, E], F32, kind="Internal")
    out = nc.dram_tensor("out", [T_CORE, H], F16, kind="ExternalOutput")

    NB = 3  # pipeline depth for per-expert buffers

    with tile.TileContext(nc) as tc, ExitStack() as ctx:
        pp = ctx.enter_context(tc.tile_pool(name="pp", bufs=1))
        xts = pp.tile([P, KT, T_CORE], F32, tag="xts")
        rwts = pp.tile([P, KT, E], F32, tag="rwts")
        bias = pp.tile([P, E], F32, tag="bias")
        wgp = pp.tile([P, MT, 64], F32, tag="wgp")       # gate table, padded to 64
        tokid = pp.tile([P, MT, E], F32, tag="tokid")    # r = 8p + j
        mv2 = pp.tile([P, E, MT], I16, tag="mv2")
        vt2 = pp.tile([16, E, 64], I16, tag="vt2")
        iotaw = pp.tile([16, 64], F32, tag="iotaw")
        neg1 = pp.tile([16, 64], I16, tag="neg1")
        nfb16 = pp.tile([16, E], F32, tag="nfb16")
        nff = pp.tile([1, E], F32, tag="nff")
        idxw = pp.tile([16, E, 64], I16, tag="idxw")
        nf = pp.tile([1, E], U32, tag="nf")
        nfc = pp.tile([1, E], U32, tag="nfc")
        idxr = pp.tile([P, E, 32], I16, tag="idxr")
        zt = pp.tile([P, H], F16, tag="zt")
        # manually rotated per-expert buffers (memset once => always "initialized")
        xg = [pp.tile([P, KT, CDMA], F16, tag=f"xg{b}", name=f"xg{b}") for b in range(NB)]
        gt = [pp.tile([P, 4, 64], F32, tag=f"gt{b}", name=f"gt{b}") for b in range(NB)]
        ys = [pp.tile([P, 4, H], F16, tag=f"ys{b}", name=f"ys{b}") for b in range(NB)]
        ws = [pp.tile([P, KT, H], F16, tag=f"ws{b}", name=f"ws{b}") for b in range(NB)]

        # ---------- prologue ----------
        nc.sync.dma_start(xts[:], xt[:])
        nc.sync.dma_start(rwts[:], rwt[:])
        nc.sync.dma_start(bias[:], rb[None, :].to_broadcast((P, E)))
        nc.vector.memset(wgp[:], 0.0)
        nc.gpsimd.iota(tokid[:], pattern=[[1, MT], [0, E]], base=0,
                       channel_multiplier=8, allow_small_or_imprecise_dtypes=True)
        nc.gpsimd.iota(iotaw[:], pattern=[[16, 64]], base=0,
                       channel_multiplier=1, allow_small_or_imprecise_dtypes=True)
        nc.vector.memset(neg1[:], -1)
        nc.vector.memset(zt[:], 0.0)
        outv = out[:].rearrange("(c p) h -> c p h", c=T_CORE // P, p=P)
        for c in range(T_CORE // P):
            nc.sync.dma_start(outv[c], zt[:])
        for b in range(NB):
            nc.vector.memset(xg[b][:], 0.0)
            nc.vector.memset(gt[b][:], 0.0)
            nc.vector.memset(ys[b][:], 0.0)

        # start streaming the first expert weights early
        nc.sync.dma_start(ws[0][:], wd[0])

        # ---------- router + gates + selection masks ----------
        with (
            tc.tile_pool(name="rpsum", bufs=2, space="PSUM") as rpsum,
            tc.tile_pool(name="small", bufs=3) as small,
        ):
            for m in range(MT):
                lps = rpsum.tile([P, E], F32, tag="lps")
                for k in range(KT):
                    nc.tensor.matmul(
                        lps[:], xts[:, k, m * P:(m + 1) * P], rwts[:, k, :],
                        start=(k == 0), stop=(k == KT - 1))
                logits = small.tile([P, E], F32, tag="logits")
                nc.vector.tensor_add(logits[:], lps[:], bias[:])
                mx8 = small.tile([P, 8], F32, tag="mx8")
                nc.vector.max(mx8[:], logits[:])
                nm = small.tile([P, 1], F32, tag="nm")
                nc.vector.tensor_scalar_mul(nm[:], mx8[:, 0:1], -1.0)
                expo = small.tile([P, E], F32, tag="expo")
                nc.scalar.activation(
                    expo[:], logits[:], mybir.ActivationFunctionType.Exp,
                    bias=nm[:], scale=1.0)
                wraw = small.tile([P, E], F32, tag="wraw")
                s6 = small.tile([P, 1], F32, tag="s6")
                nc.vector.scalar_tensor_tensor(
                    wraw[:], logits[:], mx8[:, 5:6], expo[:],
                    op0=mybir.AluOpType.is_ge, op1=mybir.AluOpType.mult,
                    accum_out=s6[:])
                r6 = small.tile([P, 1], F32, tag="r6")
                nc.vector.reciprocal(r6[:], s6[:])
                nc.vector.tensor_scalar_mul(wgp[:, m, 0:E], wraw[:], r6[:])
                # mv = sel01 * (tokid + 1) - 1   (int16 token id r, or -1)
                sel = small.tile([P, E], F32, tag="sel")
                nc.vector.tensor_scalar(
                    sel[:], logits[:], mx8[:, 5:6], None,
                    op0=mybir.AluOpType.is_ge)
                tmp = small.tile([P, E], F32, tag="tmpm")
                nc.vector.tensor_scalar_add(tmp[:], tokid[:, m, :], 1)
                nc.vector.tensor_tensor(tmp[:], sel[:], tmp[:],
                                        op=mybir.AluOpType.mult)
                nc.vector.tensor_scalar_add(mv2[:, :, m], tmp[:], -1)

        # gate table to DRAM (r-numbered rows)
        gdv = gd[:].rearrange("(q j) c -> q j c", q=P, j=MT)
        nc.sync.dma_start(gdv[:, :, :], wgp[:])

        # ---------- per-expert index lists ----------
        # vt2[pp, e, j + 8s] = mv2[16s + pp, e, j]  (wrapped pos i = 16f + pp)
        vt2v = vt2[:].rearrange("p e (s j) -> p e s j", s=8, j=MT)
        for s in range(8):
            nc.sync.dma_start(vt2v[:, :, s, :], mv2[16 * s:16 * (s + 1), :, :])
        for e in range(E):
            nc.gpsimd.sparse_gather(idxw[:, e, :], vt2[:, e, :],
                                    num_found=nf[:, e:e + 1])
        # clean HW garbage beyond each count: pos >= nf -> -1
        nc.vector.tensor_copy(nff[:], nf[:])
        nc.sync.dma_start(nfd[:], nff[:])
        nc.sync.dma_start(nfb16[:], nfd[0:1, :].to_broadcast((16, E)))
        for e in range(E):
            msk = pp.tile([16, 64], mybir.dt.uint8, tag="msk", name=f"msk{e}")
            nc.vector.tensor_scalar(msk[:], iotaw[:], nfb16[:, e:e + 1], None,
                                    op0=mybir.AluOpType.is_ge)
            nc.vector.select(idxw[:, e, :], msk[:], neg1[:], idxw[:, e, :])
        nc.vector.tensor_scalar(nfc[:], nf[:], CMM, None, op0=mybir.AluOpType.min)
        for g in range(8):
            nc.sync.dma_start(idxr[16 * g:16 * (g + 1), :, :], idxw[:, :, 0:32])

        # ---------- expert loop (software-pipelined) ----------
        with tc.tile_pool(name="mmpsum", bufs=8, space="PSUM") as mmpsum:
            cnts = [None] * E

            def issue_gathers(e):
                b = e % NB
                cnts[e] = nc.values_load(nfc[:, e:e + 1], engines=(POOL_E,),
                                         skip_runtime_bounds_check=True)
                nc.gpsimd.dma_gather(xg[b][:], xbf[:], idxr[:, e, :], CDMA,
                                     cnts[e], H, transpose=True)
                nc.gpsimd.dma_gather(gt[b][:], gd[:], idxr[:, e, :], CDMA,
                                     cnts[e], 64, transpose=False)

            issue_gathers(0)
            issue_gathers(1)
            for e in range(E):
                b = e % NB
                if e + 1 < E:
                    nc.sync.dma_start(ws[(e + 1) % NB][:], wd[e + 1])
                if e + 2 < E:
                    issue_gathers(e + 2)
                for ts in range(4):
                    M = P if ts < 3 else CMM - 3 * P
                    for nh in range(2):
                        ps = mmpsum.tile([P, 512], F32, tag="ps")
                        for k in range(KT):
                            nc.tensor.matmul(
                                ps[0:M, :], xg[b][:, k, P * ts:P * ts + M],
                                ws[b][:, k, 512 * nh:512 * (nh + 1)],
                                start=(k == 0), stop=(k == KT - 1))
                        nc.vector.tensor_scalar_mul(
                            ys[b][0:M, ts, 512 * nh:512 * (nh + 1)],
                            ps[0:M, :], gt[b][0:M, ts, e:e + 1])
                nc.gpsimd.dma_scatter_add(out[:], ys[b][:], idxr[:, e, 0:CW],
                                          CMM, cnts[e], H)

    nc.finalize()
    return nc


_PROGRAM_CACHE: dict = {}


def _get_program():
    if "p" not in _PROGRAM_CACHE:
        _PROGRAM_CACHE["p"] = build_program()
    return _PROGRAM_CACHE["p"]


# r-numbering: token t = 128j + q  <->  r = 8q + j  (q = t % 128, j = t // 128)
_R = np.arange(T_CORE)
_TOK_OF_R = 128 * (_R % 8) + _R // 8
_R_OF_TOK = np.argsort(_TOK_OF_R)


def make_core_inputs(x_core: np.ndarray, rwt: np.ndarray, rb: np.ndarray,
                     wd: np.ndarray) -> dict:
    """x_core: [1024, 1024] fp32 tokens for this core."""
    xt = np.ascontiguousarray(
        x_core.reshape(T_CORE, KT, P).transpose(2, 1, 0), dtype=np.float32)
    xbf = np.ascontiguousarray(x_core[_TOK_OF_R]).astype(np.float16)
    return {"xt": xt, "xbf": xbf, "rwt": rwt, "rb": rb, "wd": wd}


def kernel(tokens: np.ndarray, router_w: np.ndarray, router_b: np.ndarray,
           expert_w: np.ndarray) -> np.ndarray:
    from concourse.bass_utils import run_bass_kernel_spmd

    B, S, hidden = tokens.shape
    T = B * S
    assert hidden == H and T == N_CORES * T_CORE

    x = np.ascontiguousarray(tokens.reshape(T, H), dtype=np.float32)
    rwt = np.ascontiguousarray(
        router_w.T.reshape(KT, P, E).transpose(1, 0, 2)).astype(np.float32)
    rb = np.ascontiguousarray(router_b, dtype=np.float32)
    wd = np.ascontiguousarray(
        expert_w.reshape(E, KT, P, H).transpose(0, 2, 1, 3)).astype(np.float16)

    nc = _get_program()
    in_maps = [
        make_core_inputs(x[c * T_CORE:(c + 1) * T_CORE], rwt, rb, wd)
        for c in range(N_CORES)
    ]
    res = run_bass_kernel_spmd(nc, in_maps, list(range(N_CORES)))
    outs = []
    for c in range(N_CORES):
        o = np.asarray(res.results[c]["out"]).astype(np.float32)
        outs.append(o[_R_OF_TOK])          # un-permute rows to token order
    return np.concatenate(outs, axis=0).reshape(B, S, H)


# revision 29
# speedup vs baseline: 1.7655x; 1.1316x over previous
"""MoE layer (16 experts, top-6 routing, H=1024) on 8 TRN2 NeuronCores.

Strategy: data-parallel over tokens (1024 tokens/core), with SPARSE
expert compute — each token is processed only by its top-6 experts
(37.5% of the dense FLOPs) via on-device token dispatch:

  1. Router matmul (fp16 X^T tiles stationary) -> logits [128, 16] per
     token tile; top-6 selection on logits (softmax is monotone), gate
     weights renormalized over the top-6 (the reference's /acc
     renormalization collapses to exactly this).
  2. Per-expert token index lists built on-device: a selection mask in
     16-partition "wrapped" layout (8 small rearrange DMAs), then one
     gpsimd sparse_gather per expert (stream compaction), count
     clamped to capacity C=448.
  3. Per expert: dma_gather(transpose=True) pulls the selected token
     rows from DRAM as X^T tiles [128, k, C]; fp16 matmuls with the
     gathered tokens stationary and W_e moving accumulate Y = X_g @ W_e
     token-major in PSUM; DVE scales rows by the gathered gate weight;
     dma_scatter_add accumulates the scaled rows into the fp16 output
     at the token's row.

Tokens/weights/output run in fp16 (PE full speed, ~1e-4 matmul noise);
gates in fp32. Expert capacity C=448 (~4σ above the mean count of 384;
overflow tokens would be dropped gracefully). Host pre-layouts inputs
(transposes/dtype casts) and un-permutes the output; all routing math
and all FLOPs are on-device.
"""

import numpy as np
from contextlib import ExitStack

import concourse.bass as bass
import concourse.bacc as bacc
import concourse.mybir as mybir
import concourse.tile as tile

P = 128
H = 1024
E = 16
KT = 8            # h_in slices of 128
MT = 8            # token tiles of 128 per core
T_CORE = 1024
N_CORES = 8
CDMA = 512        # dma_gather transpose capacity (multiple of 128)
CMM = 448         # compute/scatter capacity (28 wrapped cols * 16)
CW = CMM // 16    # wrapped columns = 28
F16 = mybir.dt.float16
F32R = mybir.dt.float32r
F32 = mybir.dt.float32
I16 = mybir.dt.int16
U32 = mybir.dt.uint32
POOL_E = mybir.EngineType.Pool


def build_program():
    nc = bacc.Bacc(None, target_bir_lowering=False)
    # Host-prepared layouts (see kernel() below):
    #   xt[p, k, t]   = x[t, 128k + p]          fp16 (router stationary)
    #   xbf[r, h]     = x[tok_of_r[r], h]       fp16 (dispatch gather rows)
    #   rwt[p, k, e]  = router_w[e, 128k + p]   fp16
    #   rb[e]                                   fp32
    #   wd[e, p, k, h] = expert_w[e, 128k + p, h] fp16
    #   out[r, h]     (r-numbered rows; host un-permutes)
    xt = nc.dram_tensor("xt", [P, KT, T_CORE], F32, kind="ExternalInput")
    xth = nc.dram_tensor("xth", [P, KT, T_CORE], F16, kind="ExternalInput")
    xbf = nc.dram_tensor("xbf", [T_CORE, H], F16, kind="ExternalInput")
    rwt = nc.dram_tensor("rwt", [P, KT, E], F32, kind="ExternalInput")
    rb = nc.dram_tensor("rb", [E], F32, kind="ExternalInput")
    wd = nc.dram_tensor("wd", [E, P, KT, H], F16, kind="ExternalInput")
    gd = nc.dram_tensor("gd", [T_CORE, 64], F32, kind="Internal")
    mvd = nc.dram_tensor("mvd", [P, E, MT], I16, kind="Internal")
    idxd = nc.dram_tensor("idxd", [16, E, 32], I16, kind="Internal")
    out = nc.dram_tensor("out", [T_CORE, H], F16, kind="ExternalOutput")

    NB = 3  # pipeline depth for per-expert buffers

    with tile.TileContext(nc) as tc, ExitStack() as ctx:
        pp = ctx.enter_context(tc.tile_pool(name="pp", bufs=1))
        xts = pp.tile([P, KT, T_CORE], F32, tag="xts")
        xts16 = pp.tile([P, KT, T_CORE], F16, tag="xts16")
        rwts = pp.tile([P, KT, E], F32, tag="rwts")
        bias = pp.tile([P, E], F32, tag="bias")
        wgp = pp.tile([P, MT, 64], F32, tag="wgp")       # gate table, padded to 64
        tokid = pp.tile([P, MT, E], F32, tag="tokid")    # r = 8p + j
        mv2 = pp.tile([P, E, MT], I16, tag="mv2")
        vt2 = pp.tile([16, E, 64], I16, tag="vt2")
        iotaw = pp.tile([16, 64], F32, tag="iotaw")
        neg1 = pp.tile([16, E, 64], I16, tag="neg1")
        selm = pp.tile([P, MT, E], F32, tag="selm")
        ones16 = pp.tile([P, E], F32, tag="ones16")
        countT = pp.tile([16, E], F32, tag="countT")
        cntu = pp.tile([1, E], U32, tag="cntu")
        cntua = pp.tile([1, E], U32, tag="cntua")
        cntub = pp.tile([1, E], U32, tag="cntub")
        cntuc = pp.tile([1, E], U32, tag="cntuc")
        cntud = pp.tile([1, E], U32, tag="cntud")
        idxw = pp.tile([16, E, 64], I16, tag="idxw")
        nf = pp.tile([1, E], U32, tag="nf")

        idxr = pp.tile([P, E, 32], I16, tag="idxr")
        # manually rotated per-expert buffers (memset once => always "initialized")
        xg = [pp.tile([P, KT, CDMA], F16, tag=f"xg{b}", name=f"xg{b}") for b in range(NB)]
        gt = [pp.tile([P, 4, 64], F32, tag=f"gt{b}", name=f"gt{b}") for b in range(NB)]
        ys = [pp.tile([P, 4, H], F16, tag=f"ys{b}", name=f"ys{b}") for b in range(NB)]
        ws = [pp.tile([P, KT, H], F16, tag=f"ws{b}", name=f"ws{b}") for b in range(2)]
        ysd = [pp.tile([P, H], F16, tag=f"ysd{b}", name=f"ysd{b}") for b in range(2)]

        # ---------- prologue ----------
        nc.sync.dma_start(rwts[:], rwt[:])
        nc.sync.dma_start(bias[:], rb[None, :].to_broadcast((P, E)))
        for m in range(MT):
            nc.sync.dma_start(xts[:, :, m * P:(m + 1) * P],
                              xt[:, :, m * P:(m + 1) * P])
        for k in range(KT):
            nc.sync.dma_start(ws[0][:, k, :], wd[0, :, k, :])
        for m in range(MT):
            nc.sync.dma_start(xts16[:, :, m * P:(m + 1) * P],
                              xth[:, :, m * P:(m + 1) * P])
        for k in range(KT):
            nc.sync.dma_start(ws[1][:, k, :], wd[1, :, k, :])
        nc.vector.memset(wgp[:], 0.0)
        nc.gpsimd.iota(tokid[:], pattern=[[1, MT], [0, E]], base=0,
                       channel_multiplier=8, allow_small_or_imprecise_dtypes=True)
        nc.gpsimd.iota(iotaw[:], pattern=[[16, 64]], base=0,
                       channel_multiplier=1, allow_small_or_imprecise_dtypes=True)
        nc.vector.memset(neg1[:], -1)
        nc.vector.memset(ones16[:], 1.0)
        # out DRAM is pre-zeroed by the runtime (PJRT donates zeroed output
        # buffers; native run_bass_kernel_spmd pre-zeros ExternalOutputs).
        # Buffer inits on the otherwise-idle ACT engine, off the DVE path:
        nc.scalar.memzero(xg[0][:])
        nc.scalar.memzero(gt[0][:])
        for b in range(1, NB):
            nc.scalar.memzero(xg[b][:])
            nc.scalar.memzero(gt[b][:])
        for b in range(NB):
            nc.scalar.memzero(ys[b][:])



        # ---------- router + gates + selection masks ----------
        with (
            tc.tile_pool(name="rpsum", bufs=2, space="PSUM") as rpsum,
            tc.tile_pool(name="cpsum", bufs=1, space="PSUM") as cpsum,
            tc.tile_pool(name="small", bufs=3) as small,
        ):
            cps = cpsum.tile([16, E], F32, tag="cps")
            for m in range(MT):
                lps = rpsum.tile([P, E], F32, tag="lps")
                for k in range(KT):
                    nc.tensor.matmul(
                        lps[:], xts[:, k, m * P:(m + 1) * P], rwts[:, k, :],
                        start=(k == 0), stop=(k == KT - 1))
                logits = small.tile([P, E], F32, tag="logits")
                nc.vector.tensor_add(logits[:], lps[:], bias[:])
                mx8 = small.tile([P, 8], F32, tag="mx8")
                nc.vector.max(mx8[:], logits[:])
                nm = small.tile([P, 1], F32, tag="nm")
                nc.vector.tensor_scalar_mul(nm[:], mx8[:, 0:1], -1.0)
                expo = small.tile([P, E], F32, tag="expo")
                nc.scalar.activation(
                    expo[:], logits[:], mybir.ActivationFunctionType.Exp,
                    bias=nm[:], scale=1.0)
                wraw = small.tile([P, E], F32, tag="wraw")
                s6 = small.tile([P, 1], F32, tag="s6")
                nc.vector.scalar_tensor_tensor(
                    wraw[:], logits[:], mx8[:, 5:6], expo[:],
                    op0=mybir.AluOpType.is_ge, op1=mybir.AluOpType.mult,
                    accum_out=s6[:])
                r6 = small.tile([P, 1], F32, tag="r6")
                nc.vector.reciprocal(r6[:], s6[:])
                nc.vector.tensor_scalar_mul(wgp[:, m, 0:E], wraw[:], r6[:])
                # mv = sel01 * (tokid + 1) - 1   (int16 token id r, or -1)
                nc.vector.tensor_scalar(
                    selm[:, m, :], logits[:], mx8[:, 5:6], None,
                    op0=mybir.AluOpType.is_ge)
                tmp = small.tile([P, E], F32, tag="tmpm")
                nc.vector.tensor_scalar_add(tmp[:], tokid[:, m, :], 1)
                nc.vector.tensor_tensor(tmp[:], selm[:, m, :], tmp[:],
                                        op=mybir.AluOpType.mult)
                nc.vector.tensor_scalar_add(mv2[:, :, m], tmp[:], -1)
                nc.tensor.matmul(cps[:], ones16[:], selm[:, m, :],
                                 start=(m == 0), stop=(m == MT - 1))

            nc.vector.tensor_copy(countT[:], cps[:])

        # gate table to DRAM (r-numbered rows)
        gdv = gd[:].rearrange("(q j) c -> q j c", q=P, j=MT)
        nc.scalar.dma_start(gdv[:, :, :], wgp[:])

        # ---------- dense expert 0 ----------
        # Computes x @ W_0 for ALL tokens, scaled by the expert-0 gate column
        # (zero for non-selected tokens — identical math, no dispatch needed).
        # Fills the PE while the index lists are being built.
        mmpsum = ctx.enter_context(tc.tile_pool(name="mmpsum", bufs=8,
                                                space="PSUM"))
        outr = out[:].rearrange("(q j) h -> q j h", q=P, j=MT)
        for m in range(MT):
            for nh in range(2):
                ps = mmpsum.tile([P, 512], F32, tag="ps")
                for k in range(KT):
                    nc.tensor.matmul(
                        ps[:], xts16[:, k, m * P:(m + 1) * P],
                        ws[0][:, k, 512 * nh:512 * (nh + 1)],
                        start=(k == 0), stop=(k == KT - 1))
                dst = ysd[m % 2][:, 512 * nh:512 * (nh + 1)]
                if nh == 0:
                    nc.vector.tensor_scalar_mul(dst, ps[:], wgp[:, m, 0:1])
                else:
                    nc.scalar.mul(dst, ps[:], wgp[:, m, 0:1])
            nc.gpsimd.dma_start(outr[:, m, :], ysd[m % 2][:],
                                accum_op=mybir.AluOpType.add)

        # ---------- per-expert index lists ----------
        # vt2[pp, e, j + 8s] = mv2[16s + pp, e, j]  (wrapped pos i = 16f + pp)
        nc.scalar.dma_start(mvd[:], mv2[:])
        # vt2[pp, e, j + 8s] = mvd[16s + pp, e, j]
        mvdv = mvd[:].rearrange("(s pp) e j -> pp e s j", s=8, pp=16)
        nc.scalar.dma_start(vt2[:].rearrange("p e (s j) -> p e s j", s=8, j=MT),
                            mvdv)
        for e in range(E):
            nc.gpsimd.sparse_gather(idxw[:, e, :], vt2[:, e, :],
                                    num_found=nf[:, e:e + 1])
        # clean HW garbage beyond each count: pos >= count(e) -> -1 (batched)
        mskall = pp.tile([16, E, 64], mybir.dt.uint8, tag="mskall")
        nc.vector.tensor_tensor(mskall[:],
                                iotaw[:].unsqueeze(1).to_broadcast((16, E, 64)),
                                countT[:].unsqueeze(2).to_broadcast((16, E, 64)),
                                op=mybir.AluOpType.is_ge)
        nc.vector.select(idxw[:].rearrange("p e f -> p (e f)"),
                         mskall[:].rearrange("p e f -> p (e f)"),
                         neg1[:].rearrange("p e f -> p (e f)"),
                         idxw[:].rearrange("p e f -> p (e f)"))
        nc.vector.tensor_scalar(cntu[:], countT[0:1, :], CMM, None,
                                op0=mybir.AluOpType.min)
        # split-scatter counts: first 256 positions / remaining up to 448
        nc.vector.tensor_scalar(cntua[:], cntu[:], 256, None,
                                op0=mybir.AluOpType.min)
        nc.vector.tensor_tensor(cntub[:], cntu[:], cntua[:],
                                op=mybir.AluOpType.subtract)
        nc.vector.tensor_scalar(cntuc[:], cntub[:], 128, None,
                                op0=mybir.AluOpType.min)
        nc.vector.tensor_tensor(cntud[:], cntub[:], cntuc[:],
                                op=mybir.AluOpType.subtract)
        nc.scalar.dma_start(idxd[:], idxw[:, :, 0:32])
        # stream-replicate [16, E*32] -> all 8 partition groups in one DMA:
        # src iterates (g, pp, e, c) with g stride 0; dst partitions 0..127.
        nc.scalar.dma_start(
            idxr[:],
            idxd[None, :, :, :].to_broadcast((8, 16, E, 32)))

        # ---------- sparse expert loop (software-pipelined, experts 1..15) ----
        if True:
            cnts = [None] * E
            cnts_a = [None] * E
            cnts_b = [None] * E
            cnts_c = [None] * E
            cnts_d = [None] * E

            def issue_gathers(e):
                b = e % NB
                cnts[e] = nc.values_load(cntu[0:1, e:e + 1], engines=(POOL_E,),
                                          skip_runtime_bounds_check=True)
                cnts_a[e] = nc.values_load(cntua[0:1, e:e + 1], engines=(POOL_E,),
                                           skip_runtime_bounds_check=True)
                cnts_b[e] = nc.values_load(cntub[0:1, e:e + 1], engines=(POOL_E,),
                                           skip_runtime_bounds_check=True)
                if e == E - 1:
                    cnts_c[e] = nc.values_load(cntuc[0:1, e:e + 1],
                                               engines=(POOL_E,),
                                               skip_runtime_bounds_check=True)
                    cnts_d[e] = nc.values_load(cntud[0:1, e:e + 1],
                                               engines=(POOL_E,),
                                               skip_runtime_bounds_check=True)
                nc.gpsimd.dma_gather(xg[b][:], xbf[:], idxr[:, e, :], CDMA,
                                     cnts[e], H, transpose=True)
                nc.gpsimd.dma_gather(gt[b][:], gd[:], idxr[:, e, :], CDMA,
                                     cnts[e], 64, transpose=False)

            issue_gathers(1)
            issue_gathers(2)
            for e in range(1, E):
                b = e % NB
                if e + 1 < E:
                    for k in range(KT):
                        nc.sync.dma_start(ws[(e + 1) % 2][:, k, :],
                                          wd[e + 1, :, k, :])
                if e + 2 < E:
                    issue_gathers(e + 2)
                for ts in range(4):
                    M = P if ts < 3 else CMM - 3 * P
                    for nh in range(2):
                        ps = mmpsum.tile([P, 512], F32, tag="ps")
                        for k in range(KT):
                            nc.tensor.matmul(
                                ps[0:M, :], xg[b][:, k, P * ts:P * ts + M],
                                ws[e % 2][:, k, 512 * nh:512 * (nh + 1)],
                                start=(k == 0), stop=(k == KT - 1))
                        dst = ys[b][0:M, ts, 512 * nh:512 * (nh + 1)]
                        gsl = gt[b][0:M, ts, e:e + 1]
                        if (ts * 2 + nh) % 2 == 0:
                            nc.vector.tensor_scalar_mul(dst, ps[0:M, :], gsl)
                        else:
                            nc.scalar.mul(dst, ps[0:M, :], gsl)
                    if ts == 1:
                        nc.gpsimd.dma_scatter_add(
                            out[:], ys[b][:, 0:2, :], idxr[:, e, 0:16],
                            256, cnts_a[e], H)
                if e < E - 1:
                    nc.gpsimd.dma_scatter_add(
                        out[:], ys[b][:, 2:4, :], idxr[:, e, 16:CW],
                        CMM - 256, cnts_b[e], H)
                else:
                    nc.gpsimd.dma_scatter_add(
                        out[:], ys[b][:, 2:3, :], idxr[:, e, 16:24],
                        128, cnts_c[e], H)
                    nc.gpsimd.dma_scatter_add(
                        out[:], ys[b][:, 3:4, :], idxr[:, e, 24:CW],
                        CMM - 384, cnts_d[e], H)

    nc.finalize()
    return nc


_PROGRAM_CACHE: dict = {}


def _get_program():
    if "p" not in _PROGRAM_CACHE:
        _PROGRAM_CACHE["p"] = build_program()
    return _PROGRAM_CACHE["p"]


# r-numbering: token t = 128j + q  <->  r = 8q + j  (q = t % 128, j = t // 128)
_R = np.arange(T_CORE)
_TOK_OF_R = 128 * (_R % 8) + _R // 8
_R_OF_TOK = np.argsort(_TOK_OF_R)


def make_core_inputs(x_core: np.ndarray, rwt: np.ndarray, rb: np.ndarray,
                     wd: np.ndarray) -> dict:
    """x_core: [1024, 1024] fp32 tokens for this core."""
    xt = np.ascontiguousarray(
        x_core.reshape(T_CORE, KT, P).transpose(2, 1, 0), dtype=np.float32)
    xth = xt.astype(np.float16)
    xbf = np.ascontiguousarray(x_core[_TOK_OF_R]).astype(np.float16)
    return {"xt": xt, "xth": xth, "xbf": xbf, "rwt": rwt, "rb": rb, "wd": wd}


def kernel(tokens: np.ndarray, router_w: np.ndarray, router_b: np.ndarray,
           expert_w: np.ndarray) -> np.ndarray:
    from concourse.bass_utils import run_bass_kernel_spmd

    B, S, hidden = tokens.shape
    T = B * S
    assert hidden == H and T == N_CORES * T_CORE

    x = np.ascontiguousarray(tokens.reshape(T, H), dtype=np.float32)
    rwt = np.ascontiguousarray(
        router_w.T.reshape(KT, P, E).transpose(1, 0, 2)).astype(np.float32)
    rb = np.ascontiguousarray(router_b, dtype=np.float32)
    wd = np.ascontiguousarray(
        expert_w.reshape(E, KT, P, H).transpose(0, 2, 1, 3)).astype(np.float16)

    nc = _get_program()
    in_maps = [
        make_core_inputs(x[c * T_CORE:(c + 1) * T_CORE], rwt, rb, wd)
        for c in range(N_CORES)
    ]
    res = run_bass_kernel_spmd(nc, in_maps, list(range(N_CORES)))
    outs = []
    for c in range(N_CORES):
        o = np.asarray(res.results[c]["out"]).astype(np.float32)
        outs.append(o[_R_OF_TOK])          # un-permute rows to token order
    return np.concatenate(outs, axis=0).reshape(B, S, H)


# revision 30
# speedup vs baseline: 1.8421x; 1.0434x over previous
"""MoE layer (16 experts, top-6 routing, H=1024) on 8 TRN2 NeuronCores.

Strategy: data-parallel over tokens (1024 tokens/core), with SPARSE
expert compute — each token is processed only by its top-6 experts
(37.5% of the dense FLOPs) via on-device token dispatch:

  1. Router matmul (fp16 X^T tiles stationary) -> logits [128, 16] per
     token tile; top-6 selection on logits (softmax is monotone), gate
     weights renormalized over the top-6 (the reference's /acc
     renormalization collapses to exactly this).
  2. Per-expert token index lists built on-device: a selection mask in
     16-partition "wrapped" layout (8 small rearrange DMAs), then one
     gpsimd sparse_gather per expert (stream compaction), count
     clamped to capacity C=448.
  3. Per expert: dma_gather(transpose=True) pulls the selected token
     rows from DRAM as X^T tiles [128, k, C]; fp16 matmuls with the
     gathered tokens stationary and W_e moving accumulate Y = X_g @ W_e
     token-major in PSUM; DVE scales rows by the gathered gate weight;
     dma_scatter_add accumulates the scaled rows into the fp16 output
     at the token's row.

Tokens/weights/output run in fp16 (PE full speed, ~1e-4 matmul noise);
gates in fp32. Expert capacity C=448 (~4σ above the mean count of 384;
overflow tokens would be dropped gracefully). Host pre-layouts inputs
(transposes/dtype casts) and un-permutes the output; all routing math
and all FLOPs are on-device.
"""

import numpy as np
from contextlib import ExitStack

import concourse.bass as bass
import concourse.bacc as bacc
import concourse.mybir as mybir
import concourse.tile as tile

P = 128
H = 1024
E = 16
KT = 8            # h_in slices of 128
MT = 8            # token tiles of 128 per core
T_CORE = 1024
N_CORES = 8
CDMA = 512        # dma_gather transpose capacity (multiple of 128)
CMM = 448         # compute/scatter capacity (28 wrapped cols * 16)
CW = CMM // 16    # wrapped columns = 28
F16 = mybir.dt.float16
F32R = mybir.dt.float32r
F32 = mybir.dt.float32
I16 = mybir.dt.int16
U32 = mybir.dt.uint32
POOL_E = mybir.EngineType.Pool


def build_program():
    nc = bacc.Bacc(None, target_bir_lowering=False)
    # Host-prepared layouts (see kernel() below):
    #   xt[p, k, t]   = x[t, 128k + p]          fp16 (router stationary)
    #   xbf[r, h]     = x[tok_of_r[r], h]       fp16 (dispatch gather rows)
    #   rwt[p, k, e]  = router_w[e, 128k + p]   fp16
    #   rb[e]                                   fp32
    #   wd[e, p, k, h] = expert_w[e, 128k + p, h] fp16
    #   out[r, h]     (r-numbered rows; host un-permutes)
    xt = nc.dram_tensor("xt", [P, KT, T_CORE], F32, kind="ExternalInput")
    xbf = nc.dram_tensor("xbf", [T_CORE, H], F16, kind="ExternalInput")
    rwt = nc.dram_tensor("rwt", [P, KT, E], F32, kind="ExternalInput")
    rb = nc.dram_tensor("rb", [E], F32, kind="ExternalInput")
    wd = nc.dram_tensor("wd", [E, P, KT, H], F16, kind="ExternalInput")
    gd = nc.dram_tensor("gd", [T_CORE, 64], F32, kind="Internal")
    mvd = nc.dram_tensor("mvd", [P, E, MT], I16, kind="Internal")
    idxd = nc.dram_tensor("idxd", [16, E, 32], I16, kind="Internal")
    out = nc.dram_tensor("out", [T_CORE, H], F16, kind="ExternalOutput")

    NB = 3  # pipeline depth for per-expert buffers

    with tile.TileContext(nc) as tc, ExitStack() as ctx:
        pp = ctx.enter_context(tc.tile_pool(name="pp", bufs=1))
        xts = pp.tile([P, KT, T_CORE], F32, tag="xts")
        xts16 = pp.tile([P, KT, T_CORE], F16, tag="xts16")
        rwts = pp.tile([P, KT, E], F32, tag="rwts")
        bias = pp.tile([P, E], F32, tag="bias")
        wgp = pp.tile([P, MT, 64], F32, tag="wgp")       # gate table, padded to 64
        tokid = pp.tile([P, MT, E], F32, tag="tokid")    # r = 8p + j
        mv2 = pp.tile([P, E, MT], I16, tag="mv2")
        vt2 = pp.tile([16, E, 64], I16, tag="vt2")
        iotaw = pp.tile([16, 64], F32, tag="iotaw")
        neg1 = pp.tile([16, E, 64], I16, tag="neg1")
        selm = pp.tile([P, MT, E], F32, tag="selm")
        ones16 = pp.tile([P, E], F32, tag="ones16")
        countT = pp.tile([16, E], F32, tag="countT")
        cntu = pp.tile([1, E], U32, tag="cntu")
        cntua = pp.tile([1, E], U32, tag="cntua")
        cntub = pp.tile([1, E], U32, tag="cntub")
        cntuc = pp.tile([1, E], U32, tag="cntuc")
        cntud = pp.tile([1, E], U32, tag="cntud")
        idxw = pp.tile([16, E, 64], I16, tag="idxw")
        nf = pp.tile([1, E], U32, tag="nf")

        idxr = pp.tile([P, E, 32], I16, tag="idxr")
        # manually rotated per-expert buffers (memset once => always "initialized")
        xg = [pp.tile([P, KT, CDMA], F16, tag=f"xg{b}", name=f"xg{b}") for b in range(NB)]
        gt = [pp.tile([P, 4, 64], F32, tag=f"gt{b}", name=f"gt{b}") for b in range(NB)]
        ys = [pp.tile([P, 4, H], F16, tag=f"ys{b}", name=f"ys{b}") for b in range(NB)]
        ws = [pp.tile([P, KT, H], F16, tag=f"ws{b}", name=f"ws{b}") for b in range(2)]
        ysd = [pp.tile([P, H], F16, tag=f"ysd{b}", name=f"ysd{b}") for b in range(2)]

        # ---------- prologue ----------
        nc.sync.dma_start(rwts[:], rwt[:])
        nc.sync.dma_start(bias[:], rb[None, :].to_broadcast((P, E)))
        for m in range(MT):
            nc.sync.dma_start(xts[:, :, m * P:(m + 1) * P],
                              xt[:, :, m * P:(m + 1) * P])
        for k in range(KT):
            nc.sync.dma_start(ws[0][:, k, :], wd[0, :, k, :])
        for k in range(KT):
            nc.sync.dma_start(ws[1][:, k, :], wd[1, :, k, :])
        nc.vector.memset(wgp[:], 0.0)
        nc.gpsimd.iota(tokid[:], pattern=[[1, MT], [0, E]], base=0,
                       channel_multiplier=8, allow_small_or_imprecise_dtypes=True)
        nc.gpsimd.iota(iotaw[:], pattern=[[16, 64]], base=0,
                       channel_multiplier=1, allow_small_or_imprecise_dtypes=True)
        nc.vector.memset(neg1[:], -1)
        nc.vector.memset(ones16[:], 1.0)
        # out DRAM is pre-zeroed by the runtime (PJRT donates zeroed output
        # buffers; native run_bass_kernel_spmd pre-zeros ExternalOutputs).
        # Buffer inits on the otherwise-idle ACT engine, off the DVE path:
        nc.scalar.memzero(xg[0][:])
        nc.scalar.memzero(gt[0][:])
        for b in range(1, NB):
            nc.scalar.memzero(xg[b][:])
            nc.scalar.memzero(gt[b][:])
        for b in range(NB):
            nc.scalar.memzero(ys[b][:])



        # ---------- router + gates + selection masks ----------
        with (
            tc.tile_pool(name="rpsum", bufs=2, space="PSUM") as rpsum,
            tc.tile_pool(name="cpsum", bufs=1, space="PSUM") as cpsum,
            tc.tile_pool(name="small", bufs=3) as small,
        ):
            cps = cpsum.tile([16, E], F32, tag="cps")
            for m in range(MT):
                lps = rpsum.tile([P, E], F32, tag="lps")
                for k in range(KT):
                    nc.tensor.matmul(
                        lps[:], xts[:, k, m * P:(m + 1) * P], rwts[:, k, :],
                        start=(k == 0), stop=(k == KT - 1))
                nc.scalar.copy(xts16[:, :, m * P:(m + 1) * P],
                               xts[:, :, m * P:(m + 1) * P])
                logits = small.tile([P, E], F32, tag="logits")
                nc.vector.tensor_add(logits[:], lps[:], bias[:])
                mx8 = small.tile([P, 8], F32, tag="mx8")
                nc.vector.max(mx8[:], logits[:])
                nm = small.tile([P, 1], F32, tag="nm")
                nc.vector.tensor_scalar_mul(nm[:], mx8[:, 0:1], -1.0)
                expo = small.tile([P, E], F32, tag="expo")
                nc.scalar.activation(
                    expo[:], logits[:], mybir.ActivationFunctionType.Exp,
                    bias=nm[:], scale=1.0)
                wraw = small.tile([P, E], F32, tag="wraw")
                s6 = small.tile([P, 1], F32, tag="s6")
                nc.vector.scalar_tensor_tensor(
                    wraw[:], logits[:], mx8[:, 5:6], expo[:],
                    op0=mybir.AluOpType.is_ge, op1=mybir.AluOpType.mult,
                    accum_out=s6[:])
                r6 = small.tile([P, 1], F32, tag="r6")
                nc.vector.reciprocal(r6[:], s6[:])
                nc.vector.tensor_scalar_mul(wgp[:, m, 0:E], wraw[:], r6[:])
                # mv = sel01 * (tokid + 1) - 1   (int16 token id r, or -1)
                nc.vector.tensor_scalar(
                    selm[:, m, :], logits[:], mx8[:, 5:6], None,
                    op0=mybir.AluOpType.is_ge)
                tmp = small.tile([P, E], F32, tag="tmpm")
                nc.vector.tensor_scalar_add(tmp[:], tokid[:, m, :], 1)
                nc.vector.tensor_tensor(tmp[:], selm[:, m, :], tmp[:],
                                        op=mybir.AluOpType.mult)
                nc.vector.tensor_scalar_add(mv2[:, :, m], tmp[:], -1)
                nc.tensor.matmul(cps[:], ones16[:], selm[:, m, :],
                                 start=(m == 0), stop=(m == MT - 1))

            nc.vector.tensor_copy(countT[:], cps[:])

        # gate table to DRAM (r-numbered rows)
        gdv = gd[:].rearrange("(q j) c -> q j c", q=P, j=MT)
        nc.scalar.dma_start(gdv[:, :, :], wgp[:])

        # ---------- dense expert 0 ----------
        # Computes x @ W_0 for ALL tokens, scaled by the expert-0 gate column
        # (zero for non-selected tokens — identical math, no dispatch needed).
        # Fills the PE while the index lists are being built.
        mmpsum = ctx.enter_context(tc.tile_pool(name="mmpsum", bufs=8,
                                                space="PSUM"))
        outr = out[:].rearrange("(q j) h -> q j h", q=P, j=MT)
        for m in range(MT):
            for nh in range(2):
                ps = mmpsum.tile([P, 512], F32, tag="ps")
                for k in range(KT):
                    nc.tensor.matmul(
                        ps[:], xts16[:, k, m * P:(m + 1) * P],
                        ws[0][:, k, 512 * nh:512 * (nh + 1)],
                        start=(k == 0), stop=(k == KT - 1))
                dst = ysd[m % 2][:, 512 * nh:512 * (nh + 1)]
                if nh == 0:
                    nc.vector.tensor_scalar_mul(dst, ps[:], wgp[:, m, 0:1])
                else:
                    nc.scalar.mul(dst, ps[:], wgp[:, m, 0:1])
            nc.gpsimd.dma_start(outr[:, m, :], ysd[m % 2][:],
                                accum_op=mybir.AluOpType.add)

        # ---------- per-expert index lists ----------
        # vt2[pp, e, j + 8s] = mv2[16s + pp, e, j]  (wrapped pos i = 16f + pp)
        nc.scalar.dma_start(mvd[:], mv2[:])
        # vt2[pp, e, j + 8s] = mvd[16s + pp, e, j]
        mvdv = mvd[:].rearrange("(s pp) e j -> pp e s j", s=8, pp=16)
        nc.scalar.dma_start(vt2[:].rearrange("p e (s j) -> p e s j", s=8, j=MT),
                            mvdv)
        for e in range(E):
            nc.gpsimd.sparse_gather(idxw[:, e, :], vt2[:, e, :],
                                    num_found=nf[:, e:e + 1])
        # clean HW garbage beyond each count: pos >= count(e) -> -1 (batched)
        mskall = pp.tile([16, E, 64], mybir.dt.uint8, tag="mskall")
        nc.vector.tensor_tensor(mskall[:],
                                iotaw[:].unsqueeze(1).to_broadcast((16, E, 64)),
                                countT[:].unsqueeze(2).to_broadcast((16, E, 64)),
                                op=mybir.AluOpType.is_ge)
        nc.vector.select(idxw[:].rearrange("p e f -> p (e f)"),
                         mskall[:].rearrange("p e f -> p (e f)"),
                         neg1[:].rearrange("p e f -> p (e f)"),
                         idxw[:].rearrange("p e f -> p (e f)"))
        nc.vector.tensor_scalar(cntu[:], countT[0:1, :], CMM, None,
                                op0=mybir.AluOpType.min)
        # split-scatter counts: first 256 positions / remaining up to 448
        nc.vector.tensor_scalar(cntua[:], cntu[:], 256, None,
                                op0=mybir.AluOpType.min)
        nc.vector.tensor_tensor(cntub[:], cntu[:], cntua[:],
                                op=mybir.AluOpType.subtract)
        nc.vector.tensor_scalar(cntuc[:], cntub[:], 128, None,
                                op0=mybir.AluOpType.min)
        nc.vector.tensor_tensor(cntud[:], cntub[:], cntuc[:],
                                op=mybir.AluOpType.subtract)
        nc.scalar.dma_start(idxd[:], idxw[:, :, 0:32])
        # stream-replicate [16, E*32] -> all 8 partition groups in one DMA:
        # src iterates (g, pp, e, c) with g stride 0; dst partitions 0..127.
        nc.scalar.dma_start(
            idxr[:],
            idxd[None, :, :, :].to_broadcast((8, 16, E, 32)))

        # ---------- sparse expert loop (software-pipelined, experts 1..15) ----
        if True:
            cnts = [None] * E
            cnts_a = [None] * E
            cnts_b = [None] * E
            cnts_c = [None] * E
            cnts_d = [None] * E

            def issue_gathers(e):
                b = e % NB
                cnts[e] = nc.values_load(cntu[0:1, e:e + 1], engines=(POOL_E,),
                                          skip_runtime_bounds_check=True)
                cnts_a[e] = nc.values_load(cntua[0:1, e:e + 1], engines=(POOL_E,),
                                           skip_runtime_bounds_check=True)
                cnts_b[e] = nc.values_load(cntub[0:1, e:e + 1], engines=(POOL_E,),
                                           skip_runtime_bounds_check=True)
                if e == E - 1:
                    cnts_c[e] = nc.values_load(cntuc[0:1, e:e + 1],
                                               engines=(POOL_E,),
                                               skip_runtime_bounds_check=True)
                    cnts_d[e] = nc.values_load(cntud[0:1, e:e + 1],
                                               engines=(POOL_E,),
                                               skip_runtime_bounds_check=True)
                nc.gpsimd.dma_gather(xg[b][:], xbf[:], idxr[:, e, :], CDMA,
                                     cnts[e], H, transpose=True)
                nc.gpsimd.dma_gather(gt[b][:], gd[:], idxr[:, e, :], CDMA,
                                     cnts[e], 64, transpose=False)

            issue_gathers(1)
            issue_gathers(2)
            for e in range(1, E):
                b = e % NB
                if e + 1 < E:
                    for k in range(KT):
                        nc.sync.dma_start(ws[(e + 1) % 2][:, k, :],
                                          wd[e + 1, :, k, :])
                if e + 2 < E:
                    issue_gathers(e + 2)
                for ts in range(4):
                    M = P if ts < 3 else CMM - 3 * P
                    for nh in range(2):
                        ps = mmpsum.tile([P, 512], F32, tag="ps")
                        for k in range(KT):
                            nc.tensor.matmul(
                                ps[0:M, :], xg[b][:, k, P * ts:P * ts + M],
                                ws[e % 2][:, k, 512 * nh:512 * (nh + 1)],
                                start=(k == 0), stop=(k == KT - 1))
                        dst = ys[b][0:M, ts, 512 * nh:512 * (nh + 1)]
                        gsl = gt[b][0:M, ts, e:e + 1]
                        if (ts * 2 + nh) % 2 == 0:
                            nc.vector.tensor_scalar_mul(dst, ps[0:M, :], gsl)
                        else:
                            nc.scalar.mul(dst, ps[0:M, :], gsl)
                    if ts == 1:
                        nc.gpsimd.dma_scatter_add(
                            out[:], ys[b][:, 0:2, :], idxr[:, e, 0:16],
                            256, cnts_a[e], H)
                if e < E - 1:
                    nc.gpsimd.dma_scatter_add(
                        out[:], ys[b][:, 2:4, :], idxr[:, e, 16:CW],
                        CMM - 256, cnts_b[e], H)
                else:
                    nc.gpsimd.dma_scatter_add(
                        out[:], ys[b][:, 2:3, :], idxr[:, e, 16:24],
                        128, cnts_c[e], H)
                    nc.gpsimd.dma_scatter_add(
                        out[:], ys[b][:, 3:4, :], idxr[:, e, 24:CW],
                        CMM - 384, cnts_d[e], H)

    nc.finalize()
    return nc


_PROGRAM_CACHE: dict = {}


def _get_program():
    if "p" not in _PROGRAM_CACHE:
        _PROGRAM_CACHE["p"] = build_program()
    return _PROGRAM_CACHE["p"]


# r-numbering: token t = 128j + q  <->  r = 8q + j  (q = t % 128, j = t // 128)
_R = np.arange(T_CORE)
_TOK_OF_R = 128 * (_R % 8) + _R // 8
_R_OF_TOK = np.argsort(_TOK_OF_R)


def make_core_inputs(x_core: np.ndarray, rwt: np.ndarray, rb: np.ndarray,
                     wd: np.ndarray) -> dict:
    """x_core: [1024, 1024] fp32 tokens for this core."""
    xt = np.ascontiguousarray(
        x_core.reshape(T_CORE, KT, P).transpose(2, 1, 0), dtype=np.float32)
    xbf = np.ascontiguousarray(x_core[_TOK_OF_R]).astype(np.float16)
    return {"xt": xt, "xbf": xbf, "rwt": rwt, "rb": rb, "wd": wd}


def kernel(tokens: np.ndarray, router_w: np.ndarray, router_b: np.ndarray,
           expert_w: np.ndarray) -> np.ndarray:
    from concourse.bass_utils import run_bass_kernel_spmd

    B, S, hidden = tokens.shape
    T = B * S
    assert hidden == H and T == N_CORES * T_CORE

    x = np.ascontiguousarray(tokens.reshape(T, H), dtype=np.float32)
    rwt = np.ascontiguousarray(
        router_w.T.reshape(KT, P, E).transpose(1, 0, 2)).astype(np.float32)
    rb = np.ascontiguousarray(router_b, dtype=np.float32)
    wd = np.ascontiguousarray(
        expert_w.reshape(E, KT, P, H).transpose(0, 2, 1, 3)).astype(np.float16)

    nc = _get_program()
    in_maps = [
        make_core_inputs(x[c * T_CORE:(c + 1) * T_CORE], rwt, rb, wd)
        for c in range(N_CORES)
    ]
    res = run_bass_kernel_spmd(nc, in_maps, list(range(N_CORES)))
    outs = []
    for c in range(N_CORES):
        o = np.asarray(res.results[c]["out"]).astype(np.float32)
        outs.append(o[_R_OF_TOK])          # un-permute rows to token order
    return np.concatenate(outs, axis=0).reshape(B, S, H)
